# revision 12
# baseline (speedup 1.0000x reference)
"""BatchedRoutingLinear kernel for 8 TRN2 NeuronCores (Bass/Tile).

out = x @ W.T, with bias added at the top-32 rows of W by cosine similarity
(per batch row). Weight/bias are sharded row-wise over out_dim across the 8
cores; x is replicated. Per-core local top-32 candidate values are AllGathered
and every core recomputes the global 32nd-largest value per batch row as a
threshold; the bias scatter-add is then an exact thresholded mask-add on the
local shard (top-32 of the union == global top-32, no ties for iid normal
data).

Layout: the host passes W already transposed ([in_dim, out_dim_local],
contiguous), so weight tiles land in SBUF with the contraction dim on
partitions and the PE runs plain LDWEIGHTS+MATMUL pairs — no on-device
transposes of the big operand. Row norms and their reciprocals are tiny
derived inputs ([out_dim] fp32) also prepared host-side.
"""

import numpy as np

from concourse import bass, mybir
from concourse import tile as _tile_mod
from concourse.bass_utils import run_bass_kernel_spmd
from concourse.masks import make_identity
from concourse.tile import TileContext
from concourse.vector_clock import ScopedClock, VectorClock

F32 = mybir.dt.float32

# ---------------------------------------------------------------------------
# TileContext tail-drain patch: stock _drain_and_barrier hangs every
# outstanding sem wait on one Drain, which exceeds this walrus build's
# per-instruction sync-wait limit. Split waits one-logical-proc-per-NOP, then
# emit a wait-free drain (SP is in-order, so the drain still observes them).
# ---------------------------------------------------------------------------
_N_PROCS = 27


def _patched_drain_and_barrier(self, tick_clock, wait_clock):
    full = tick_clock.global_clock
    for j in range(_N_PROCS):
        if full[j] == 0:
            continue
        partial = VectorClock([full[p] if p == j else 0 for p in range(_N_PROCS)])
        nop = self.nc.sync.nop(nofuse=True, hint=f"drain_split_{j}")
        wait_clock.add_sem_waits(nop.ins, ScopedClock({None: partial}))
    self.nc.sync.drain()
    self.nc.all_engine_barrier()
    assert self.sems is not None
    popped = self.nc._tile_sem_poison_stack.pop()
    assert popped is self._sem_poison
    self.nc.clear_and_free_semaphores(list(self.sems.allocated().values()))
    self.nc.all_engine_barrier()


_tile_mod.TileContext._drain_and_barrier = _patched_drain_and_barrier

# ---------------------------------------------------------------------------
# Second walrus workaround: this neuronxcc build accepts at most ONE sync wait
# per instruction, but Tile's semaphore assigner freely attaches several.
# Intercept every instruction at commit time and spill all-but-one wait onto
# single-wait NoOps on the same engine, placed immediately before it.
# ---------------------------------------------------------------------------
_orig_commit = _tile_mod.TileContext._commit_instruction
_spill_counter = [0]


def _split_excess_waits(self, inst):
    si = getattr(inst, "sync_info", None)
    if si is None or not si.on_wait or len(si.on_wait) <= 1:
        return
    waits = list(si.on_wait)
    for w in waits[:-1]:
        _spill_counter[0] += 1
        nop = mybir.InstNoOp(
            name=f"waitspill_{_spill_counter[0]}",
            sync_info=mybir.SyncInfo(on_wait=[w], on_update=[]),
            bass_nofuse=True,
            engine=inst.engine,
        )
        _orig_commit(self, nop)
    inst.sync_info = mybir.SyncInfo(on_wait=[waits[-1]], on_update=list(si.on_update))


def _patched_commit(self, inst, lazy_reg_writes=True):
    _split_excess_waits(self, inst)
    return _orig_commit(self, inst, lazy_reg_writes)


_tile_mod.TileContext._commit_instruction = _patched_commit

# ---------------------------------------------------------------------------
# Problem constants (hardcoded per spec).
# ---------------------------------------------------------------------------
N_CORES = 8
B = 8
D = 512
O_FULL = 128000
TOP_K = 32
O_LOCAL = O_FULL // N_CORES  # 16000
NT = O_LOCAL // 128          # 125 weight tiles of 128 rows per core
DC = D // 128                # 4 contraction chunks
NEG = -3.0e38                # "-inf" for match_replace masking
OUT_GROUPS = [(g * 16, min(16, NT - g * 16)) for g in range((NT + 15) // 16)]


def _topk_rounds(nc, vals, scratch, out32, k_rounds):
    """Extract top-(8*k_rounds) of `vals` [P, n] into out32 [P, 8*k_rounds]
    (descending within each round of 8). `scratch` is clobbered."""
    src = vals
    for r in range(k_rounds):
        nc.vector.max(out=out32[:, r * 8:(r + 1) * 8], in_=src)
        if r < k_rounds - 1:
            nc.vector.match_replace(
                out=scratch,
                in_to_replace=out32[:, r * 8:(r + 1) * 8],
                in_values=src,
                imm_value=NEG,
            )
            src = scratch


def build_kernel():
    nc = bass.Bass(num_devices=N_CORES)

    xT_d = nc.dram_tensor("xT", [D, B], F32, kind="ExternalInput").ap()
    wT_d = nc.dram_tensor("wT", [D, O_LOCAL], F32, kind="ExternalInput").ap()
    bias_d = nc.dram_tensor("bias_pc", [128, NT], F32, kind="ExternalInput").ap()
    norm_d = nc.dram_tensor("norm_pc", [128, NT], F32, kind="ExternalInput").ap()
    inorm_d = nc.dram_tensor("inorm_pc", [128, NT], F32, kind="ExternalInput").ap()
    out_d = nc.dram_tensor("out", [B, O_LOCAL], F32, kind="ExternalOutput").ap()

    with TileContext(nc) as tc:
        with (
            tc.tile_pool(name="const", bufs=1) as const_pool,
            tc.tile_pool(name="wt", bufs=4) as wt_pool,
            tc.tile_pool(name="acc", bufs=1) as acc_pool,
            tc.tile_pool(name="dram", bufs=1, space="DRAM") as dram_pool,
        ):
            ident = const_pool.tile([128, 128], F32)
            make_identity(nc, ident)

            xT = const_pool.tile([128, DC, B], F32)
            nc.sync.dma_start(
                out=xT, in_=xT_d.rearrange("(c p) b -> p c b", p=128)
            )
            bias_sb = const_pool.tile([128, NT], F32)
            nc.sync.dma_start(out=bias_sb, in_=bias_d)
            norm_sb = const_pool.tile([128, NT], F32)
            nc.sync.dma_start(out=norm_sb, in_=norm_d)
            inorm_sb = const_pool.tile([128, NT], F32)
            nc.sync.dma_start(out=inorm_sb, in_=inorm_d)

            # sim accumulator over the whole local shard: sim[p, t*8+b]
            sim_sb = acc_pool.tile([128, NT * B], F32)

            # ---------------- streaming phase ----------------
            stream_psum_d = tc.tile_pool(name="pd", bufs=4, space="PSUM")
            pd_pool = stream_psum_d.__enter__()
            wT_view = wT_d.rearrange("(c p) o -> p c o", p=128)
            for t in range(NT):
                wt = wt_pool.tile([128, DC, 128], F32)
                dma_eng = nc.sync if t % 2 == 0 else nc.scalar
                dma_eng.dma_start(
                    out=wt, in_=wT_view[:, :, t * 128:(t + 1) * 128]
                )

                pdots = pd_pool.tile([128, B], F32)
                for c in range(DC):
                    nc.tensor.matmul(
                        pdots,
                        lhsT=wt[:, c, :],
                        rhs=xT[:, c, :],
                        start=(c == 0),
                        stop=(c == DC - 1),
                    )

                # sim = dots / ||w||  (per-partition scalar multiply)
                nc.vector.tensor_scalar(
                    out=sim_sb[:, t * B:(t + 1) * B],
                    in0=pdots,
                    scalar1=inorm_sb[:, t:t + 1],
                    scalar2=None,
                    op0=mybir.AluOpType.mult,
                )

            stream_psum_d.__exit__(None, None, None)

            # ---------------- top-k phase ----------------
            sim3 = sim_sb.rearrange("p (c b) -> p c b", b=B)

            with (
                tc.tile_pool(name="tail", bufs=1) as tail_pool,
                tc.tile_pool(name="po", bufs=2, space="PSUM") as po_pool,
            ):
                # per-partition top-8 candidates for each batch row
                cand8 = tail_pool.tile([128, B * 8], F32)
                for b in range(B):
                    nc.vector.max(
                        out=cand8[:, b * 8:(b + 1) * 8], in_=sim3[:, :, b]
                    )

                # gather candidates to [B, 128*8] (one partition per batch row)
                candb = tail_pool.tile([B, 128 * 8], F32)
                for b in range(B):
                    nc.sync.dma_start(
                        out=candb[b:b + 1, :],
                        in_=cand8[:, b * 8:(b + 1) * 8],
                    )

                # local top-32 per batch row
                local32 = tail_pool.tile([B, TOP_K], F32)
                cscratch = tail_pool.tile([B, 128 * 8], F32)
                _topk_rounds(nc, candb, cscratch, local32, TOP_K // 8)

                # AllGather local top-32 values across the 8 cores
                cc_in = dram_pool.tile([B, TOP_K], F32)
                cc_out = dram_pool.tile(
                    [N_CORES * B, TOP_K], F32, addr_space="Shared"
                )
                nc.sync.dma_start(out=cc_in, in_=local32)
                nc.gpsimd.collective_compute(
                    "AllGather",
                    mybir.AluOpType.bypass,
                    replica_groups=[list(range(N_CORES))],
                    ins=[cc_in.opt()],
                    outs=[cc_out.opt()],
                )

                # global 32nd-largest value per row = threshold t_b
                allv = tail_pool.tile([B, N_CORES * TOP_K], F32)
                nc.sync.dma_start(
                    out=allv,
                    in_=cc_out.rearrange("(k b) j -> b k j", b=B),
                )
                th32 = tail_pool.tile([B, TOP_K], F32)
                ascratch = tail_pool.tile([B, N_CORES * TOP_K], F32)
                _topk_rounds(nc, allv, ascratch, th32, TOP_K // 8)

                # broadcast t_b to all 128 partitions: tcols[p, b] = t_b
                # (K=1 matmul against a ones row — outer product broadcast)
                trow = tail_pool.tile([1, B], F32)
                nc.sync.dma_start(out=trow, in_=th32[:, TOP_K - 1:TOP_K])
                ones_row = tail_pool.tile([1, 128], F32)
                nc.vector.memset(ones_row, 1.0)
                ptc = po_pool.tile([128, B], F32, tag="ptc", bufs=1)
                nc.tensor.matmul(
                    ptc, lhsT=ones_row, rhs=trow, start=True, stop=True
                )
                tcols = tail_pool.tile([128, B], F32)
                nc.scalar.copy(out=tcols, in_=ptc)

                # final = sim * norm + bias * (sim >= t_b)
                mask = tail_pool.tile([128, NT * B], F32)
                mask3 = mask.rearrange("p (c b) -> p c b", b=B)
                final = tail_pool.tile([128, NT * B], F32)
                final3 = final.rearrange("p (c b) -> p c b", b=B)
                for b in range(B):
                    nc.vector.tensor_scalar(
                        out=mask3[:, :, b],
                        in0=sim3[:, :, b],
                        scalar1=tcols[:, b:b + 1],
                        scalar2=None,
                        op0=mybir.AluOpType.is_ge,
                    )
                    nc.vector.tensor_tensor(
                        out=mask3[:, :, b],
                        in0=mask3[:, :, b],
                        in1=bias_sb,
                        op=mybir.AluOpType.mult,
                    )
                    nc.vector.tensor_tensor(
                        out=final3[:, :, b],
                        in0=sim3[:, :, b],
                        in1=norm_sb,
                        op=mybir.AluOpType.mult,
                    )
                nc.vector.tensor_tensor(
                    out=final, in0=final, in1=mask, op=mybir.AluOpType.add
                )

                # transpose back to [b, o] and store
                for g, (c0, gw) in enumerate(OUT_GROUPS):
                    pout = po_pool.tile([128, 128], F32)
                    osb = tail_pool.tile(
                        [128, 128], F32, tag="osb", bufs=2
                    )
                    nc.tensor.transpose(
                        pout[:gw * B, :],
                        final[:, c0 * B:(c0 + gw) * B],
                        ident,
                    )
                    if g % 2 == 0:
                        nc.scalar.copy(out=osb[:gw * B, :], in_=pout[:gw * B, :])
                    else:
                        nc.vector.tensor_copy(
                            out=osb[:gw * B, :], in_=pout[:gw * B, :]
                        )
                    nc.sync.dma_start(
                        out=out_d[:, c0 * 128:(c0 + gw) * 128].rearrange(
                            "b (c o) -> c b o", o=128
                        ),
                        in_=osb[:gw * B, :],
                    )

    return nc


_NC_CACHE = None


def _host_prep(x, weight, bias):
    """Per-core input maps: transposed weight shard, packed bias/norms."""
    xT = np.ascontiguousarray(x.T)  # [D, B]
    in_maps = []
    for k in range(N_CORES):
        wsh = weight[k * O_LOCAL:(k + 1) * O_LOCAL]       # [O_LOCAL, D]
        wT = np.ascontiguousarray(wsh.T)                  # [D, O_LOCAL]
        norms = np.sqrt(
            np.sum(wsh.astype(np.float32) ** 2, axis=1, dtype=np.float32)
        ).astype(np.float32)                              # [O_LOCAL]
        bsh = bias[k * O_LOCAL:(k + 1) * O_LOCAL]
        in_maps.append(
            {
                "xT": xT,
                "wT": wT,
                "bias_pc": np.ascontiguousarray(bsh.reshape(NT, 128).T),
                "norm_pc": np.ascontiguousarray(norms.reshape(NT, 128).T),
                "inorm_pc": np.ascontiguousarray(
                    (np.float32(1.0) / norms).reshape(NT, 128).T
                ),
            }
        )
    return in_maps


def kernel(x, weight, bias, top_k):
    global _NC_CACHE
    assert int(top_k) == TOP_K
    x = np.ascontiguousarray(np.asarray(x, dtype=np.float32))
    weight = np.ascontiguousarray(np.asarray(weight, dtype=np.float32))
    bias = np.ascontiguousarray(np.asarray(bias, dtype=np.float32))

    if _NC_CACHE is None:
        _NC_CACHE = build_kernel()
    nc = _NC_CACHE

    in_maps = _host_prep(x, weight, bias)
    res = run_bass_kernel_spmd(nc, in_maps, core_ids=list(range(N_CORES)))
    out = np.concatenate(
        [res.results[k]["out"] for k in range(N_CORES)], axis=1
    )
    return out.astype(np.float32)
